# revision 44
# baseline (speedup 1.0000x reference)
"""AttentionConv kernel for Trainium2 (8 NeuronCores, SPMD data-parallel over batch).

Problem: per-channel windowed softmax attention.
  q = wq @ x; k = wk @ pad(x, 3); v = wv @ pad(x, 3)       (1x1 convs = GEMMs)
  s_j[c,w] = q[c,w] * k[c,w+j],  j = 0..6
  out[c,w] = sum_j softmax_j(s)[c,w,j] * v[c,w+j]

Sharding: batch B=8 -> one batch element per core; weights replicated.

v3.3 (chunk-granular pipeline; ~103us vs 115us v2 baseline):
  - every intermediate (q/k/v/scores) lives in per-(co, m) 1024-col chunk
    tiles (k/v carry 3-col halos filled by mini-evacs), so the
    tile-coarse dependency tracker chains everything at chunk level:
    scores chase GEMM evacs, dens chase scores, evs chase dens, nums
    chase evs, finals chase num evacs
  - dep-free PE warmup (garbage matmuls on an st tile) releases the HAM
    clock-gate before the first real GEMM; no DMA interaction
  - host packs x as [128, 4, 2, GG]; one DMA per (chunk, ci-half),
    feed-ordered and split across both HWDGE queues (sync + scalar)
  - k/q GEMMs first (in score-gating order, v deferred) so DVE starts
    at ~14us; v GEMMs interleave with den sums on PE afterwards
  - exp: 6 planes via DVE Schraudolph tensor_scalar (4x int16 bit-trick,
    N_TS=6), 1 plane exact on ACT - balances the DVE/ACT load
  - den/num 7-plane sums on PE as identity matmuls (deduped LDWEIGHTS),
    j-outer h-inner so consecutive MMs alternate PSUM banks
  - finals interleaved 2 behind the ev stream, trailing ones read num
    PSUM directly; last chunk split 768+256 so the trailing PE num (and
    the ~5us end-of-kernel PE drain behind it) starts earliest
  - ACT table patch pins Copy/Exp/Ln to one table set: single load
"""

import sys

sys.path.insert(0, "/opt/trn_rl_repo")

import numpy as np

B, C, W = 8, 256, 4096
K7, PAD = 7, 3
GG = 1024  # gemm group / chunk width
SC = 1024  # sum-chunk width (= GG)
KW = SC + 2 * PAD  # per-(co,m) k/v tile width (1030)
N_CH = W // SC  # 4 chunks per co block

# --- tuning knobs -----------------------------------------------------------
SCHRAUD_C0 = 184.6650390625  # 2^7 / ln 2
SCHRAUD_C1 = 16250.0  # 127 * 128 - sigma
N_WARM = 40  # dep-free PE warmup matmuls (N=128 each)
N_TS = 6  # planes 0..N_TS-1 exp'd via DVE Schraudolph; rest exact on ACT
N_DIRECT = 2  # trailing finals that read num PSUM directly (skip ACT evac)

_STATE = {}


def _patch_act_tables():
    """Pin Copy, Exp and Ln to the one ACT table set containing all three,
    so the kernel pays a single ACT_TABLE_LOAD."""
    import concourse.bacc as bacc_mod
    import concourse.mybir as mybir
    from concourse.hw_specs import get_activation_tables as orig

    AF = mybir.ActivationFunctionType

    def patched(arch):
        out = {}
        for name, funcs in orig(arch).items():
            f = set(funcs)
            if name != "natural_log_exp_and_others":
                f.discard(AF.Exp)
                f.discard(AF.Ln)
                f.discard(AF.Copy)
            out[name] = f
        return out

    bacc_mod.get_activation_tables = patched


def _build_nc():
    import concourse.bass as bass
    import concourse.tile as tile
    from concourse import bacc, mybir

    _patch_act_tables()

    bf16 = mybir.dt.bfloat16
    i16 = mybir.dt.int16
    f32 = mybir.dt.float32
    AF = mybir.ActivationFunctionType
    ALU = mybir.AluOpType

    nc = bacc.Bacc("TRN2", target_bir_lowering=False, debug=False, num_devices=8)

    # x packed [128, chunk, ci_half, GG]: 4KB contiguous per partition/chunk
    x_d = nc.declare_dram_parameter("x", [128, N_CH, 2, GG], bf16, isOutput=False)
    w_d = {
        t: nc.declare_dram_parameter(f"wt{t}", [128, 2, C], bf16, isOutput=False)
        for t in "qkv"
    }
    id_d = nc.declare_dram_parameter("ident", [128, 128], bf16, isOutput=False)
    out_d = nc.declare_dram_parameter("out", [C, W], bf16, isOutput=True)

    with tile.TileContext(nc) as tc:
        from contextlib import ExitStack

        with ExitStack() as ctx:
            persist = ctx.enter_context(tc.tile_pool(name="persist", bufs=1))
            lpool = ctx.enter_context(tc.tile_pool(name="lpool", bufs=1))

            # ---- persistent SBUF tensors ----
            wsb = {
                t: persist.tile([128, 2, C], bf16, name=f"wsb_{t}", tag=f"wsb_{t}")
                for t in "qkv"
            }  # w.T halves: [ci_part, ci_half, co]
            idt = persist.tile([128, 128], bf16, tag="idt")
            rdsb = persist.tile([128, 2, W], bf16, tag="rdsb")

            def grid(mk, nm):
                return [
                    [mk(co, m, f"{nm}{co}{m}") for m in range(N_CH)] for co in range(2)
                ]

            # k/v per (co, m) with 3-col halos; q and scores per (co, m)
            ksb = grid(lambda co, m, n: persist.tile([128, KW], bf16, name=n, tag=n), "k")
            vsb = grid(lambda co, m, n: persist.tile([128, KW], bf16, name=n, tag=n), "v")
            st = grid(
                lambda co, m, n: persist.tile([128, K7, SC], bf16, name=n, tag=n), "s"
            )

            def mm(out, lhsT, rhs, start, stop, **kw):
                return nc.tensor.matmul(out, lhsT, rhs, start=start, stop=stop, **kw)

            def halo_memsets():
                for buf in (ksb, vsb):
                    for co in range(2):
                        nc.vector.memset(buf[co][0][:, 0:PAD], 0.0)
                        nc.vector.memset(buf[co][N_CH - 1][:, KW - PAD : KW], 0.0)

            def warmup(ppool):
                """PE warmup burst on garbage st data so HAM releases before
                the first real GEMM. No DMA deps at all."""
                wps = ppool.tile([128, GG], f32, name="wps", tag="gps")
                for _ in range(N_WARM):
                    mm(
                        wps[:, 0:128],
                        st[0][0][:, 0, 0:128],
                        st[0][0][:, 0, 0:128],
                        start=True,
                        stop=True,
                        skip_group_check=True,
                    )

            def gemm_group(co, g, t, ppool, xb, qsb, eng="act"):
                """GEMM of tensor t, cols [g*GG, (g+1)*GG); evacuation into
                chunk tiles (k/v: main + 3-col halo minis into neighbours).
                eng picks the main-evac engine (DVE only for the head k's)."""
                co_sl = slice(co * 128, (co + 1) * 128)
                ps = ppool.tile([128, GG], f32, name="ps", tag="gps")
                for ci in range(2):
                    for i in range(GG // 512):
                        mm(
                            ps[:, i * 512 : (i + 1) * 512],
                            wsb[t][:, ci, co_sl],
                            xb[g][ci][:, i * 512 : (i + 1) * 512],
                            start=(ci == 0),
                            stop=(ci == 1),
                        )
                copy = (
                    (lambda o, i_: nc.vector.tensor_copy(out=o, in_=i_))
                    if eng == "dve"
                    else (lambda o, i_: nc.scalar.copy(out=o, in_=i_))
                )
                if t == "q" and eng == "act512":
                    nc.scalar.copy(out=qsb[co][g][:, 0:512], in_=ps[:, 0:512])
                    nc.scalar.copy(out=qsb[co][g][:, 512:GG], in_=ps[:, 512:GG])
                    return
                if t == "k" and eng == "dve512":
                    # split so the first score half only waits the first piece
                    nc.vector.tensor_copy(
                        out=ksb[co][g][:, PAD : PAD + 515], in_=ps[:, 0:515]
                    )
                    nc.vector.tensor_copy(
                        out=ksb[co][g][:, PAD + 515 : PAD + GG], in_=ps[:, 515:GG]
                    )
                    nc.scalar.copy(
                        out=ksb[co][g + 1][:, 0:PAD], in_=ps[:, GG - PAD : GG]
                    )
                    return
                if t == "q":
                    copy(qsb[co][g][:, :], ps[:, :])
                else:
                    buf = ksb if t == "k" else vsb
                    # evac g covers padded cols [3 + g*GG, 3 + (g+1)*GG):
                    #   main -> tile[g][3:1027]
                    #   first 3 cols -> tile[g-1][1027:1030]  (g >= 1)
                    #   last 3 cols -> tile[g+1][0:3]         (g <= 2)
                    if g >= 1:
                        nc.scalar.copy(
                            out=buf[co][g - 1][:, KW - PAD : KW], in_=ps[:, 0:PAD]
                        )
                    copy(buf[co][g][:, PAD : PAD + GG], ps[:, :])
                    if g <= 2:
                        nc.scalar.copy(
                            out=buf[co][g + 1][:, 0:PAD], in_=ps[:, GG - PAD : GG]
                        )

            # per-chunk exp split: all-Schraudolph where ACT is congested
            # (head chunks pace on kq evacs), exact ACT planes where it idles
            N_TS_MAP = {
                (0, 0): 7, (1, 0): 7, (0, 1): 7, (1, 1): 6,
                (0, 2): 6, (1, 2): 5, (0, 3): 5, (1, 3): 5,
            }

            def scores_exp(co, m, qsb, h0=0, hw=SC):
                """s = q*k (7-plane TT, bf16 2x) then Schraudolph exp in place
                (TS 4x int16 bit-trick) for cols [h0, h0+hw) of a chunk."""
                n_ts = N_TS_MAP[(co, m)]
                dst = st[co][m][:, :, h0 : h0 + hw]
                qsl = qsb[co][m][:, :]
                ksl = ksb[co][m][:, :]
                q_b = bass.AP(
                    tensor=qsl.tensor,
                    offset=qsl.offset + h0,
                    ap=[qsl.ap[0], [0, K7], [1, hw]],
                )
                k_w = bass.AP(
                    tensor=ksl.tensor,
                    offset=ksl.offset + h0,
                    ap=[ksl.ap[0], [1, K7], [1, hw]],
                )
                nc.vector.tensor_tensor(dst, q_b, k_w, ALU.mult)
                ts = st[co][m][:, 0:n_ts, h0 : h0 + hw]
                nc.vector.tensor_scalar(
                    ts.bitcast(i16), ts, SCHRAUD_C0, SCHRAUD_C1, ALU.mult, ALU.add
                )
                if n_ts < K7:
                    ex = st[co][m][:, n_ts:K7, h0 : h0 + hw]
                    nc.scalar.activation(ex, ex, AF.Exp)

            def ev_mult(co, m, h0=0, hw=SC):
                """ev_j = e_j * v_j in place for cols [h0, h0+hw) of a chunk."""
                sl = st[co][m][:, :, h0 : h0 + hw]
                vsl = vsb[co][m][:, :]
                vw = bass.AP(
                    tensor=vsl.tensor,
                    offset=vsl.offset + h0,
                    ap=[vsl.ap[0], [1, K7], [1, hw]],
                )
                nc.vector.tensor_tensor(sl, sl, vw, ALU.mult)

            def pe_sum_sub(co, m, h0, hw, pool):
                """7-plane identity-matmul sum for cols [h0, h0+hw);
                matmuls tiled to the 512-col fp32 PSUM limit."""
                ps = pool.tile([128, hw], f32, name="acch", tag="acc")
                for j in range(K7):
                    for b0 in range(0, hw, 512):
                        bw = min(512, hw - b0)
                        mm(
                            ps[:, b0 : b0 + bw],
                            idt[:, :],
                            st[co][m][:, j, h0 + b0 : h0 + b0 + bw],
                            start=(j == 0),
                            stop=(j == K7 - 1),
                            skip_group_check=True,
                        )
                return ps

            def fin_nsb(co, m, opool):
                """final for a chunk whose num halves are already in nsb."""
                co_sl = slice(co * 128, (co + 1) * 128)
                w0 = m * SC
                rd = rdsb[:, co, w0 : w0 + SC]
                oc = opool.tile([128, SC], bf16, name="oc", tag="oc")
                nc.vector.tensor_tensor(
                    oc[:, :], nsb[co][:, w0 : w0 + SC], rd, ALU.mult
                )
                _out_dma(out_d[co_sl, w0 : w0 + SC], oc[:, :])

            def final_sub(co, m, h0, hw, nump, opool):
                co_sl = slice(co * 128, (co + 1) * 128)
                w0 = m * SC + h0
                rd = rdsb[:, co, w0 : w0 + hw]
                oc = opool.tile([128, hw], bf16, name="och", tag="oc")
                nc.vector.tensor_tensor(oc[:, :], nump[:, :], rd, ALU.mult)
                _out_dma(out_d[co_sl, w0 : w0 + hw], oc[:, :])

            def pe_sum(co, m, pool, tag="acc"):
                """7-plane sum over j via identity matmuls into PSUM.
                j-outer / h-inner so consecutive MMs alternate PSUM banks."""
                ps = pool.tile([128, SC], f32, name="acc", tag=tag)
                for j in range(K7):
                    for h in range(SC // 512):
                        mm(
                            ps[:, h * 512 : (h + 1) * 512],
                            idt[:, :],
                            st[co][m][:, j, h * 512 : (h + 1) * 512],
                            start=(j == 0),
                            stop=(j == K7 - 1),
                            skip_group_check=True,
                        )
                return ps

            def rden_of(denp, co, m, lpool):
                """rden = exp(-ln(den)) on ACT into persistent rdsb."""
                t = lpool.tile([128, SC], f32, name="lnt", tag="lnt")
                r = rdsb[:, co, m * SC : (m + 1) * SC]
                nc.scalar.activation(t[:, :], denp[:, :], AF.Ln)
                nc.scalar.activation(r, t[:, :], AF.Exp, scale=-1.0)
                return r

            _fin_q = [0]

            def _out_dma(dst, src_):
                _fin_q[0] ^= 1
                eng = nc.sync if _fin_q[0] else nc.scalar
                eng.dma_start(out=dst, in_=src_)

            def final_out(co, m, nump, nsb, opool, direct=False):
                co_sl = slice(co * 128, (co + 1) * 128)
                w0 = m * SC
                rd = rdsb[:, co, w0 : w0 + SC]
                oc = opool.tile([128, SC], bf16, name="oc", tag="oc")
                if direct:
                    nc.vector.tensor_tensor(oc[:, :], nump[:, :], rd, ALU.mult)
                else:
                    ns = nsb[co][:, w0 : w0 + SC]
                    nc.scalar.copy(out=ns, in_=nump[:, :])
                    nc.vector.tensor_tensor(oc[:, :], ns, rd, ALU.mult)
                _out_dma(out_d[co_sl, w0 : w0 + SC], oc[:, :])

            # ---- emission ----
            with (
                tc.tile_pool(name="xpool", bufs=1) as xpool,
                tc.tile_pool(name="ppool", bufs=4, space="PSUM") as ppool,
            ):
                xb = [
                    [
                        xpool.tile([128, GG], bf16, name=f"xb{g}{ci}", tag=f"xb{g}{ci}")
                        for ci in range(2)
                    ]
                    for g in range(N_CH)
                ]
                qsb = grid(
                    lambda co, m, n: xpool.tile([128, SC], bf16, name=n, tag=n), "q"
                )

                halo_memsets()
                warmup(ppool)

                # DMA triggers: feed-ordered, split across the HWDGE queues
                nc.sync.dma_start(out=xb[0][0][:, :], in_=x_d[:, 0, 0, :])
                nc.sync.dma_start(out=wsb["q"][:, :, :], in_=w_d["q"][:, :, :])
                nc.sync.dma_start(out=xb[1][0][:, :], in_=x_d[:, 1, 0, :])
                nc.sync.dma_start(out=xb[2][0][:, :], in_=x_d[:, 2, 0, :])
                nc.sync.dma_start(out=xb[2][1][:, :], in_=x_d[:, 2, 1, :])
                nc.sync.dma_start(out=idt[:, :], in_=id_d[:, :])
                nc.scalar.dma_start(out=wsb["k"][:, :, :], in_=w_d["k"][:, :, :])
                nc.scalar.dma_start(out=xb[0][1][:, :], in_=x_d[:, 0, 1, :])
                nc.scalar.dma_start(out=xb[1][1][:, :], in_=x_d[:, 1, 1, :])
                nc.scalar.dma_start(out=wsb["v"][:, :, :], in_=w_d["v"][:, :, :])
                nc.scalar.dma_start(out=xb[3][0][:, :], in_=x_d[:, 3, 0, :])
                nc.scalar.dma_start(out=xb[3][1][:, :], in_=x_d[:, 3, 1, :])

                # k/q GEMMs feed-ordered so scores unblock earliest; the two
                # head k evacs ride the idle DVE; scores chase per chunk
                kq_order = [
                    [("k", 0, 0, "dve512"), ("q", 0, 0, "act512"), ("k", 0, 1, "act"),
                     ("k", 1, 0, "act"), ("q", 1, 0, "act"), ("k", 1, 1, "act")],
                    [("k", 0, 2, "act"), ("q", 0, 1, "act"),
                     ("k", 1, 2, "act"), ("q", 1, 1, "act")],
                    [("k", 0, 3, "act"), ("q", 0, 2, "act"),
                     ("k", 1, 3, "act"), ("q", 1, 2, "act")],
                    [("q", 0, 3, "act"), ("q", 1, 3, "act")],
                ]
                for mi, blk in enumerate(kq_order):
                    for t, co, g, eng in blk:
                        gemm_group(co, g, t, ppool, xb, qsb, eng)
                        if mi == 0 and (t, co) == ("q", 0):
                            scores_exp(0, 0, qsb, 0, 512)
                        if mi == 0 and (t, co, g) == ("k", 0, 1):
                            scores_exp(0, 0, qsb, 512, 512)
                    if mi == 0:
                        scores_exp(1, 0, qsb)
                    else:
                        scores_exp(0, mi, qsb)
                        scores_exp(1, mi, qsb)

                # v GEMMs interleaved with dens (PE) + rdens (ACT)
                for g in range(N_CH):
                    gemm_group(0, g, "v", ppool, xb, qsb)
                    gemm_group(1, g, "v", ppool, xb, qsb)
                    for co in range(2):
                        denp = pe_sum(co, g, ppool, "gps")
                        rden_of(denp, co, g, lpool)

            late = ctx.enter_context(tc.tile_pool(name="late", bufs=1))
            opool = ctx.enter_context(tc.tile_pool(name="opool", bufs=6))
            nsb = [
                late.tile([128, W], bf16, name=f"nsb{co}", tag=f"nsb{co}")
                for co in range(2)
            ]
            with tc.tile_pool(name="apsum", bufs=4, space="PSUM") as apsum:
                order = [(co, m) for m in range(N_CH) for co in range(2)]
                nums = {}
                for i, (co, m) in enumerate(order[:-1]):
                    ev_mult(co, m)
                    nums[(co, m)] = pe_sum_sub(co, m, 0, SC, apsum)
                    if i >= 2:
                        fco, fm = order[i - 2]
                        final_out(fco, fm, nums[(fco, fm)], nsb, opool)
                final_out(*order[-3], nums[order[-3]], nsb, opool, True)
                final_out(*order[-2], nums[order[-2]], nsb, opool, True)
                lco, lm = order[-1]
                ev_mult(lco, lm, 0, 768)
                ps7 = pe_sum_sub(lco, lm, 0, 768, apsum)
                ev_mult(lco, lm, 768, 256)
                ps2 = pe_sum_sub(lco, lm, 768, 256, apsum)
                final_sub(lco, lm, 0, 768, ps7, opool)
                final_sub(lco, lm, 768, 256, ps2, opool)

    _dedupe_ldweights(nc, mybir)
    nc.finalize()
    return nc


def _dedupe_ldweights(nc, mybir):
    """Remove redundant InstLdweights: a reload of the exact weights already
    resident in the PE array. Only drops loads that carry no semaphore
    waits/updates; any other PE instruction type resets the signature."""

    def wsig(ldw):
        return (
            str(ldw.ins[0]),
            str(ldw.is_transpose),
            str(ldw.perf_mode),
            str(ldw.tile_position),
        )

    removed = 0
    for f in nc.m.functions:
        for b in f.blocks:
            keep = []
            last = None
            for i in b.instructions:
                tn = type(i).__name__
                if getattr(i, "engine", None) == mybir.EngineType.PE:
                    if tn == "InstLdweights":
                        si = i.sync_info
                        clean = si is None or (
                            len(si.on_wait) == 0 and len(si.on_update) == 0
                        )
                        if clean and last == wsig(i):
                            removed += 1
                            continue
                        last = wsig(i)
                    elif tn in ("InstMatmult", "InstEventSemaphore"):
                        pass
                    else:
                        last = None
                keep.append(i)
            b.instructions[:] = keep
    return removed


def _get_nc():
    if "nc" not in _STATE:
        _STATE["nc"] = _build_nc()
    return _STATE["nc"]


def _pack2(a):
    """[256, N] -> [128, 2, N] with [p, h, n] = a[h*128 + p, n]."""
    return np.ascontiguousarray(a.reshape(2, 128, -1).transpose(1, 0, 2))


def _pack_x(a):
    """[256, W] -> [128, n_ch, 2, GG] with [p, g, h, n] = a[h*128+p, g*GG+n]."""
    n_ch = a.shape[1] // GG
    b = a.reshape(2, 128, n_ch, GG)
    return np.ascontiguousarray(b.transpose(1, 2, 0, 3))


def _make_in_maps(x, wq, wk, wv):
    import ml_dtypes

    bf = ml_dtypes.bfloat16

    x = np.asarray(x, dtype=np.float32)
    wqT = np.ascontiguousarray(np.asarray(wq, dtype=np.float32).T).astype(bf)
    wkT = np.ascontiguousarray(np.asarray(wk, dtype=np.float32).T).astype(bf)
    wvT = np.ascontiguousarray(np.asarray(wv, dtype=np.float32).T).astype(bf)
    xb = x.astype(bf)
    ident = np.eye(128, dtype=np.float32).astype(bf)

    wq2, wk2, wv2 = _pack2(wqT), _pack2(wkT), _pack2(wvT)
    return [
        {
            "x": _pack_x(xb[b]),
            "wtq": wq2,
            "wtk": wk2,
            "wtv": wv2,
            "ident": ident,
        }
        for b in range(B)
    ]


def kernel(x, wq, wk, wv):
    nc = _get_nc()
    in_maps = _make_in_maps(x, wq, wk, wv)

    from concourse.bass_utils import run_bass_kernel_spmd

    res = run_bass_kernel_spmd(nc, in_maps, core_ids=list(range(B)))
    outs = [np.asarray(res.results[i]["out"], dtype=np.float32) for i in range(B)]
    return np.stack(outs)


# revision 45
# speedup vs baseline: 1.0082x; 1.0082x over previous
"""AttentionConv kernel for Trainium2 (8 NeuronCores, SPMD data-parallel over batch).

Problem: per-channel windowed softmax attention.
  q = wq @ x; k = wk @ pad(x, 3); v = wv @ pad(x, 3)       (1x1 convs = GEMMs)
  s_j[c,w] = q[c,w] * k[c,w+j],  j = 0..6
  out[c,w] = sum_j softmax_j(s)[c,w,j] * v[c,w+j]

Sharding: batch B=8 -> one batch element per core; weights replicated.

v3.3 (chunk-granular pipeline; ~103us vs 115us v2 baseline):
  - every intermediate (q/k/v/scores) lives in per-(co, m) 1024-col chunk
    tiles (k/v carry 3-col halos filled by mini-evacs), so the
    tile-coarse dependency tracker chains everything at chunk level:
    scores chase GEMM evacs, dens chase scores, evs chase dens, nums
    chase evs, finals chase num evacs
  - dep-free PE warmup (garbage matmuls on an st tile) releases the HAM
    clock-gate before the first real GEMM; no DMA interaction
  - host packs x as [128, 4, 2, GG]; one DMA per (chunk, ci-half),
    feed-ordered and split across both HWDGE queues (sync + scalar)
  - k/q GEMMs first (in score-gating order, v deferred) so DVE starts
    at ~14us; v GEMMs interleave with den sums on PE afterwards
  - exp: 6 planes via DVE Schraudolph tensor_scalar (4x int16 bit-trick,
    N_TS=6), 1 plane exact on ACT - balances the DVE/ACT load
  - den/num 7-plane sums on PE as identity matmuls (deduped LDWEIGHTS),
    j-outer h-inner so consecutive MMs alternate PSUM banks
  - finals interleaved 2 behind the ev stream, trailing ones read num
    PSUM directly; last chunk split 768+256 so the trailing PE num (and
    the ~5us end-of-kernel PE drain behind it) starts earliest
  - ACT table patch pins Copy/Exp/Ln to one table set: single load
"""

import sys

sys.path.insert(0, "/opt/trn_rl_repo")

import numpy as np

B, C, W = 8, 256, 4096
K7, PAD = 7, 3
GG = 1024  # gemm group / chunk width
SC = 1024  # sum-chunk width (= GG)
KW = SC + 2 * PAD  # per-(co,m) k/v tile width (1030)
N_CH = W // SC  # 4 chunks per co block

# --- tuning knobs -----------------------------------------------------------
SCHRAUD_C0 = 184.6650390625  # 2^7 / ln 2
SCHRAUD_C1 = 16250.0  # 127 * 128 - sigma
N_WARM = 40  # dep-free PE warmup matmuls (N=128 each)
N_TS = 6  # planes 0..N_TS-1 exp'd via DVE Schraudolph; rest exact on ACT
N_DIRECT = 2  # trailing finals that read num PSUM directly (skip ACT evac)

_STATE = {}


def _patch_act_tables():
    """Pin Copy, Exp and Ln to the one ACT table set containing all three,
    so the kernel pays a single ACT_TABLE_LOAD."""
    import concourse.bacc as bacc_mod
    import concourse.mybir as mybir
    from concourse.hw_specs import get_activation_tables as orig

    AF = mybir.ActivationFunctionType

    def patched(arch):
        out = {}
        for name, funcs in orig(arch).items():
            f = set(funcs)
            if name != "natural_log_exp_and_others":
                f.discard(AF.Exp)
                f.discard(AF.Ln)
                f.discard(AF.Copy)
            out[name] = f
        return out

    bacc_mod.get_activation_tables = patched


def _build_nc():
    import concourse.bass as bass
    import concourse.tile as tile
    from concourse import bacc, mybir

    _patch_act_tables()

    bf16 = mybir.dt.bfloat16
    i16 = mybir.dt.int16
    f32 = mybir.dt.float32
    AF = mybir.ActivationFunctionType
    ALU = mybir.AluOpType

    nc = bacc.Bacc("TRN2", target_bir_lowering=False, debug=False, num_devices=8)

    # x packed [128, chunk, ci_half, GG]: 4KB contiguous per partition/chunk
    x_d = nc.declare_dram_parameter("x", [128, N_CH, 2, GG], bf16, isOutput=False)
    w_d = {
        t: nc.declare_dram_parameter(f"wt{t}", [128, 2, C], bf16, isOutput=False)
        for t in "qkv"
    }
    id_d = nc.declare_dram_parameter("ident", [128, 128], bf16, isOutput=False)
    out_d = nc.declare_dram_parameter("out", [C, W], bf16, isOutput=True)

    with tile.TileContext(nc) as tc:
        from contextlib import ExitStack

        with ExitStack() as ctx:
            persist = ctx.enter_context(tc.tile_pool(name="persist", bufs=1))
            lpool = ctx.enter_context(tc.tile_pool(name="lpool", bufs=1))

            # ---- persistent SBUF tensors ----
            wsb = {
                t: persist.tile([128, 2, C], bf16, name=f"wsb_{t}", tag=f"wsb_{t}")
                for t in "qkv"
            }  # w.T halves: [ci_part, ci_half, co]
            idt = persist.tile([128, 128], bf16, tag="idt")
            rdsb = persist.tile([128, 2, W], bf16, tag="rdsb")

            def grid(mk, nm):
                return [
                    [mk(co, m, f"{nm}{co}{m}") for m in range(N_CH)] for co in range(2)
                ]

            # k/v per (co, m) with 3-col halos; q and scores per (co, m)
            ksb = grid(lambda co, m, n: persist.tile([128, KW], bf16, name=n, tag=n), "k")
            vsb = grid(lambda co, m, n: persist.tile([128, KW], bf16, name=n, tag=n), "v")
            st = grid(
                lambda co, m, n: persist.tile([128, K7, SC], bf16, name=n, tag=n), "s"
            )

            def mm(out, lhsT, rhs, start, stop, **kw):
                return nc.tensor.matmul(out, lhsT, rhs, start=start, stop=stop, **kw)

            def halo_memsets():
                for buf in (ksb, vsb):
                    for co in range(2):
                        nc.vector.memset(buf[co][0][:, 0:PAD], 0.0)
                        nc.vector.memset(buf[co][N_CH - 1][:, KW - PAD : KW], 0.0)

            def warmup(ppool):
                """PE warmup burst on garbage st data so HAM releases before
                the first real GEMM. No DMA deps at all."""
                wps = ppool.tile([128, GG], f32, name="wps", tag="gps")
                for _ in range(N_WARM):
                    mm(
                        wps[:, 0:128],
                        st[0][0][:, 0, 0:128],
                        st[0][0][:, 0, 0:128],
                        start=True,
                        stop=True,
                        skip_group_check=True,
                    )

            def gemm_group(co, g, t, ppool, xb, qsb, eng="act"):
                """GEMM of tensor t, cols [g*GG, (g+1)*GG); evacuation into
                chunk tiles (k/v: main + 3-col halo minis into neighbours).
                eng picks the main-evac engine (DVE only for the head k's)."""
                co_sl = slice(co * 128, (co + 1) * 128)
                ps = ppool.tile([128, GG], f32, name="ps", tag="gps")
                for ci in range(2):
                    for i in range(GG // 512):
                        mm(
                            ps[:, i * 512 : (i + 1) * 512],
                            wsb[t][:, ci, co_sl],
                            xb[g][ci][:, i * 512 : (i + 1) * 512],
                            start=(ci == 0),
                            stop=(ci == 1),
                        )
                copy = (
                    (lambda o, i_: nc.vector.tensor_copy(out=o, in_=i_))
                    if eng == "dve"
                    else (lambda o, i_: nc.scalar.copy(out=o, in_=i_))
                )
                if t == "q" and eng == "act512":
                    nc.scalar.copy(out=qsb[co][g][:, 0:512], in_=ps[:, 0:512])
                    nc.scalar.copy(out=qsb[co][g][:, 512:GG], in_=ps[:, 512:GG])
                    return
                if t == "k" and eng == "dve512":
                    # split so the first score half only waits the first piece
                    nc.vector.tensor_copy(
                        out=ksb[co][g][:, PAD : PAD + 515], in_=ps[:, 0:515]
                    )
                    nc.vector.tensor_copy(
                        out=ksb[co][g][:, PAD + 515 : PAD + GG], in_=ps[:, 515:GG]
                    )
                    nc.scalar.copy(
                        out=ksb[co][g + 1][:, 0:PAD], in_=ps[:, GG - PAD : GG]
                    )
                    return
                if t == "q":
                    copy(qsb[co][g][:, :], ps[:, :])
                else:
                    buf = ksb if t == "k" else vsb
                    # evac g covers padded cols [3 + g*GG, 3 + (g+1)*GG):
                    #   main -> tile[g][3:1027]
                    #   first 3 cols -> tile[g-1][1027:1030]  (g >= 1)
                    #   last 3 cols -> tile[g+1][0:3]         (g <= 2)
                    if g >= 1:
                        nc.scalar.copy(
                            out=buf[co][g - 1][:, KW - PAD : KW], in_=ps[:, 0:PAD]
                        )
                    copy(buf[co][g][:, PAD : PAD + GG], ps[:, :])
                    if g <= 2:
                        nc.scalar.copy(
                            out=buf[co][g + 1][:, 0:PAD], in_=ps[:, GG - PAD : GG]
                        )

            # per-chunk exp split: all-Schraudolph where ACT is congested
            # (head chunks pace on kq evacs), exact ACT planes where it idles
            N_TS_MAP = {
                (0, 0): 7, (1, 0): 7, (0, 1): 7, (1, 1): 6,
                (0, 2): 6, (1, 2): 5, (0, 3): 5, (1, 3): 5,
            }

            def scores_exp(co, m, qsb, h0=0, hw=SC):
                """s = q*k (7-plane TT, bf16 2x) then Schraudolph exp in place
                (TS 4x int16 bit-trick) for cols [h0, h0+hw) of a chunk."""
                n_ts = N_TS_MAP[(co, m)]
                dst = st[co][m][:, :, h0 : h0 + hw]
                qsl = qsb[co][m][:, :]
                ksl = ksb[co][m][:, :]
                q_b = bass.AP(
                    tensor=qsl.tensor,
                    offset=qsl.offset + h0,
                    ap=[qsl.ap[0], [0, K7], [1, hw]],
                )
                k_w = bass.AP(
                    tensor=ksl.tensor,
                    offset=ksl.offset + h0,
                    ap=[ksl.ap[0], [1, K7], [1, hw]],
                )
                nc.vector.tensor_tensor(dst, q_b, k_w, ALU.mult)
                ts = st[co][m][:, 0:n_ts, h0 : h0 + hw]
                nc.vector.tensor_scalar(
                    ts.bitcast(i16), ts, SCHRAUD_C0, SCHRAUD_C1, ALU.mult, ALU.add
                )
                if n_ts < K7:
                    ex = st[co][m][:, n_ts:K7, h0 : h0 + hw]
                    nc.scalar.activation(ex, ex, AF.Exp)

            def ev_mult(co, m, h0=0, hw=SC):
                """ev_j = e_j * v_j in place for cols [h0, h0+hw) of a chunk."""
                sl = st[co][m][:, :, h0 : h0 + hw]
                vsl = vsb[co][m][:, :]
                vw = bass.AP(
                    tensor=vsl.tensor,
                    offset=vsl.offset + h0,
                    ap=[vsl.ap[0], [1, K7], [1, hw]],
                )
                nc.vector.tensor_tensor(sl, sl, vw, ALU.mult)

            def pe_sum_sub(co, m, h0, hw, pool):
                """7-plane identity-matmul sum for cols [h0, h0+hw);
                matmuls tiled to the 512-col fp32 PSUM limit."""
                ps = pool.tile([128, hw], f32, name="acch", tag="acc")
                for j in range(K7):
                    for b0 in range(0, hw, 512):
                        bw = min(512, hw - b0)
                        mm(
                            ps[:, b0 : b0 + bw],
                            idt[:, :],
                            st[co][m][:, j, h0 + b0 : h0 + b0 + bw],
                            start=(j == 0),
                            stop=(j == K7 - 1),
                            skip_group_check=True,
                        )
                return ps

            def fin_nsb(co, m, opool):
                """final for a chunk whose num halves are already in nsb."""
                co_sl = slice(co * 128, (co + 1) * 128)
                w0 = m * SC
                rd = rdsb[:, co, w0 : w0 + SC]
                oc = opool.tile([128, SC], bf16, name="oc", tag="oc")
                nc.vector.tensor_tensor(
                    oc[:, :], nsb[co][:, w0 : w0 + SC], rd, ALU.mult
                )
                _out_dma(out_d[co_sl, w0 : w0 + SC], oc[:, :])

            def final_sub(co, m, h0, hw, nump, opool):
                co_sl = slice(co * 128, (co + 1) * 128)
                w0 = m * SC + h0
                rd = rdsb[:, co, w0 : w0 + hw]
                oc = opool.tile([128, hw], bf16, name="och", tag="oc")
                nc.vector.tensor_tensor(oc[:, :], nump[:, :], rd, ALU.mult)
                _out_dma(out_d[co_sl, w0 : w0 + hw], oc[:, :])

            def pe_sum(co, m, pool, tag="acc"):
                """7-plane sum over j via identity matmuls into PSUM.
                j-outer / h-inner so consecutive MMs alternate PSUM banks."""
                ps = pool.tile([128, SC], f32, name="acc", tag=tag)
                for j in range(K7):
                    for h in range(SC // 512):
                        mm(
                            ps[:, h * 512 : (h + 1) * 512],
                            idt[:, :],
                            st[co][m][:, j, h * 512 : (h + 1) * 512],
                            start=(j == 0),
                            stop=(j == K7 - 1),
                            skip_group_check=True,
                        )
                return ps

            def rden_of(denp, co, m, lpool):
                """rden = exp(-ln(den)) on ACT into persistent rdsb."""
                t = lpool.tile([128, SC], f32, name="lnt", tag="lnt")
                r = rdsb[:, co, m * SC : (m + 1) * SC]
                nc.scalar.activation(t[:, :], denp[:, :], AF.Ln)
                nc.scalar.activation(r, t[:, :], AF.Exp, scale=-1.0)
                return r

            _fin_q = [0]

            def _out_dma(dst, src_):
                _fin_q[0] ^= 1
                eng = nc.sync if _fin_q[0] else nc.scalar
                eng.dma_start(out=dst, in_=src_)

            def final_out(co, m, nump, nsb, opool, direct=False):
                co_sl = slice(co * 128, (co + 1) * 128)
                w0 = m * SC
                rd = rdsb[:, co, w0 : w0 + SC]
                oc = opool.tile([128, SC], bf16, name="oc", tag="oc")
                if direct:
                    nc.vector.tensor_tensor(oc[:, :], nump[:, :], rd, ALU.mult)
                else:
                    ns = nsb[co][:, w0 : w0 + SC]
                    nc.scalar.copy(out=ns, in_=nump[:, :])
                    nc.vector.tensor_tensor(oc[:, :], ns, rd, ALU.mult)
                _out_dma(out_d[co_sl, w0 : w0 + SC], oc[:, :])

            # ---- emission ----
            with (
                tc.tile_pool(name="xpool", bufs=1) as xpool,
                tc.tile_pool(name="ppool", bufs=4, space="PSUM") as ppool,
            ):
                xb = [
                    [
                        xpool.tile([128, GG], bf16, name=f"xb{g}{ci}", tag=f"xb{g}{ci}")
                        for ci in range(2)
                    ]
                    for g in range(N_CH)
                ]
                qsb = grid(
                    lambda co, m, n: xpool.tile([128, SC], bf16, name=n, tag=n), "q"
                )

                halo_memsets()
                warmup(ppool)

                # DMA triggers: feed-ordered, split across the HWDGE queues
                nc.sync.dma_start(out=xb[0][0][:, :], in_=x_d[:, 0, 0, :])
                nc.sync.dma_start(out=wsb["q"][:, :, :], in_=w_d["q"][:, :, :])
                nc.sync.dma_start(out=xb[1][0][:, :], in_=x_d[:, 1, 0, :])
                nc.sync.dma_start(out=xb[2][0][:, :], in_=x_d[:, 2, 0, :])
                nc.sync.dma_start(out=xb[2][1][:, :], in_=x_d[:, 2, 1, :])
                nc.sync.dma_start(out=idt[:, :], in_=id_d[:, :])
                nc.scalar.dma_start(out=wsb["k"][:, :, :], in_=w_d["k"][:, :, :])
                nc.scalar.dma_start(out=xb[0][1][:, :], in_=x_d[:, 0, 1, :])
                nc.scalar.dma_start(out=xb[1][1][:, :], in_=x_d[:, 1, 1, :])
                nc.scalar.dma_start(out=wsb["v"][:, :, :], in_=w_d["v"][:, :, :])
                nc.scalar.dma_start(out=xb[3][0][:, :], in_=x_d[:, 3, 0, :])
                nc.scalar.dma_start(out=xb[3][1][:, :], in_=x_d[:, 3, 1, :])

                # k/q GEMMs feed-ordered so scores unblock earliest; the two
                # head k evacs ride the idle DVE; scores chase per chunk
                kq_order = [
                    [("k", 0, 0, "dve512"), ("q", 0, 0, "act512"), ("k", 0, 1, "act"),
                     ("k", 1, 0, "act"), ("q", 1, 0, "act"), ("k", 1, 1, "act")],
                    [("k", 0, 2, "act"), ("q", 0, 1, "act"),
                     ("k", 1, 2, "act"), ("q", 1, 1, "act")],
                    [("k", 0, 3, "act"), ("q", 0, 2, "act"),
                     ("k", 1, 3, "act"), ("q", 1, 2, "act")],
                    [("q", 0, 3, "act"), ("q", 1, 3, "act")],
                ]
                for mi, blk in enumerate(kq_order):
                    for t, co, g, eng in blk:
                        gemm_group(co, g, t, ppool, xb, qsb, eng)
                        if mi == 0 and (t, co) == ("q", 0):
                            scores_exp(0, 0, qsb, 0, 512)
                        if mi == 0 and (t, co, g) == ("k", 0, 1):
                            scores_exp(0, 0, qsb, 512, 512)
                    if mi == 0:
                        scores_exp(1, 0, qsb)
                    else:
                        scores_exp(0, mi, qsb)
                        scores_exp(1, mi, qsb)

                # v GEMMs interleaved with dens (PE) + rdens (ACT)
                for g in range(N_CH):
                    gemm_group(0, g, "v", ppool, xb, qsb)
                    gemm_group(1, g, "v", ppool, xb, qsb)
                    for co in range(2):
                        denp = pe_sum(co, g, ppool, "gps")
                        rden_of(denp, co, g, lpool)

            late = ctx.enter_context(tc.tile_pool(name="late", bufs=1))
            opool = ctx.enter_context(tc.tile_pool(name="opool", bufs=4))
            nsb = [
                late.tile([128, W], bf16, name=f"nsb{co}", tag=f"nsb{co}")
                for co in range(2)
            ]
            with tc.tile_pool(name="apsum", bufs=4, space="PSUM") as apsum:
                order = [(co, m) for m in range(N_CH) for co in range(2)]
                nums = {}
                for i, (co, m) in enumerate(order[:-1]):
                    ev_mult(co, m)
                    nums[(co, m)] = pe_sum_sub(co, m, 0, SC, apsum)
                    if i >= 2:
                        fco, fm = order[i - 2]
                        final_out(fco, fm, nums[(fco, fm)], nsb, opool)
                final_out(*order[-3], nums[order[-3]], nsb, opool, True)
                final_out(*order[-2], nums[order[-2]], nsb, opool, True)
                lco, lm = order[-1]
                ev_mult(lco, lm, 0, 768)
                ps7 = pe_sum_sub(lco, lm, 0, 768, apsum)
                ev_mult(lco, lm, 768, 256)
                ps2 = pe_sum_sub(lco, lm, 768, 256, apsum)
                final_sub(lco, lm, 0, 768, ps7, opool)
                final_sub(lco, lm, 768, 256, ps2, opool)

    _dedupe_ldweights(nc, mybir)
    nc.finalize()
    return nc


def _dedupe_ldweights(nc, mybir):
    """Remove redundant InstLdweights: a reload of the exact weights already
    resident in the PE array. Only drops loads that carry no semaphore
    waits/updates; any other PE instruction type resets the signature."""

    def wsig(ldw):
        return (
            str(ldw.ins[0]),
            str(ldw.is_transpose),
            str(ldw.perf_mode),
            str(ldw.tile_position),
        )

    removed = 0
    for f in nc.m.functions:
        for b in f.blocks:
            keep = []
            last = None
            for i in b.instructions:
                tn = type(i).__name__
                if getattr(i, "engine", None) == mybir.EngineType.PE:
                    if tn == "InstLdweights":
                        si = i.sync_info
                        clean = si is None or (
                            len(si.on_wait) == 0 and len(si.on_update) == 0
                        )
                        if clean and last == wsig(i):
                            removed += 1
                            continue
                        last = wsig(i)
                    elif tn in ("InstMatmult", "InstEventSemaphore"):
                        pass
                    else:
                        last = None
                keep.append(i)
            b.instructions[:] = keep
    return removed


def _get_nc():
    if "nc" not in _STATE:
        _STATE["nc"] = _build_nc()
    return _STATE["nc"]


def _pack2(a):
    """[256, N] -> [128, 2, N] with [p, h, n] = a[h*128 + p, n]."""
    return np.ascontiguousarray(a.reshape(2, 128, -1).transpose(1, 0, 2))


def _pack_x(a):
    """[256, W] -> [128, n_ch, 2, GG] with [p, g, h, n] = a[h*128+p, g*GG+n]."""
    n_ch = a.shape[1] // GG
    b = a.reshape(2, 128, n_ch, GG)
    return np.ascontiguousarray(b.transpose(1, 2, 0, 3))


def _make_in_maps(x, wq, wk, wv):
    import ml_dtypes

    bf = ml_dtypes.bfloat16

    x = np.asarray(x, dtype=np.float32)
    wqT = np.ascontiguousarray(np.asarray(wq, dtype=np.float32).T).astype(bf)
    wkT = np.ascontiguousarray(np.asarray(wk, dtype=np.float32).T).astype(bf)
    wvT = np.ascontiguousarray(np.asarray(wv, dtype=np.float32).T).astype(bf)
    xb = x.astype(bf)
    ident = np.eye(128, dtype=np.float32).astype(bf)

    wq2, wk2, wv2 = _pack2(wqT), _pack2(wkT), _pack2(wvT)
    return [
        {
            "x": _pack_x(xb[b]),
            "wtq": wq2,
            "wtk": wk2,
            "wtv": wv2,
            "ident": ident,
        }
        for b in range(B)
    ]


def kernel(x, wq, wk, wv):
    nc = _get_nc()
    in_maps = _make_in_maps(x, wq, wk, wv)

    from concourse.bass_utils import run_bass_kernel_spmd

    res = run_bass_kernel_spmd(nc, in_maps, core_ids=list(range(B)))
    outs = [np.asarray(res.results[i]["out"], dtype=np.float32) for i in range(B)]
    return np.stack(outs)
